# revision 15
# baseline (speedup 1.0000x reference)
"""Trainium2 Bass kernel for nn_CodeExpressionContextMixer.

Computes, for a mapping (key -> val) over AST/CFG node tables:
    u   = tanh(cfg[val] @ W_update + b_update)
    z   = sigmoid(prev[key] @ Wg1 + u @ Wg2 + b_gate)
    out = prev.at[key].set(z * prev[key] + (1 - z) * u)

Strategy (8 NeuronCores, SPMD, no collectives):
  * The index shuffle is host-side prep: pairs are split evenly across
    cores and the host gathers prev[key] / cfg[val] into dense fp16
    feature-major arrays per core.  The device kernel is pure dense
    streaming: no on-device gathers, no masks - the host scatters the
    (disjoint) target rows back and passes unmapped rows through
    exactly.
  * u matmuls and the prev side of the gate run fp16.  The u side of
    the K=512 gate matmul runs fp8e4m3 with DoubleRow (2 weights per
    PE cell, one pass instead of two): the tanh activation writes u
    as fp8 directly, so the cast is free.
  * Gate weights/bias are negated so ACT computes zp = 1-z =
    sigmoid(-arg); the device ships delta = zp*(u-p) in fp16 and the
    host adds the f32 prev rows back (better precision than routing
    prev through fp16, and one less DVE op per tile).
"""

import os
import numpy as np

R = 500000          # AST rows
CFGN = 100000       # CFG rows
D = 256             # feature dim
NCORES = 8
M = 400000          # mapped pairs
P = 50176           # padded pairs per core (49 * 1024)
W = 2048            # steady-state supertile width (cols)
U_FP8 = True        # u as fp8 (DoubleRow gate) vs all-fp16 fallback

_cache = {}


def _build(zero_bias, u_fp8):
    key = (zero_bias, u_fp8)
    if key in _cache:
        return _cache[key]
    from contextlib import ExitStack
    import concourse.bass as bass
    import concourse.tile as tile
    from concourse import bacc, mybir

    F32 = mybir.dt.float32
    F16 = mybir.dt.float16
    F8 = mybir.dt.float8e4
    FU = F8 if u_fp8 else F16
    AF = mybir.ActivationFunctionType
    DR = mybir.MatmulPerfMode.DoubleRow

    nc = bacc.Bacc("TRN2", target_bir_lowering=False, debug=False)

    prevT = nc.dram_tensor("prevT", [D, P], F16, kind="ExternalInput").ap()
    ctxT = nc.dram_tensor("ctxT", [D, P], F16, kind="ExternalInput").ap()
    wu = nc.dram_tensor("wu", [D, D], F16, kind="ExternalInput").ap()
    wgn = nc.dram_tensor("wgn", [2 * D, D], F16, kind="ExternalInput").ap()
    # fp8 u-side gate weights, DoubleRow layout [ki, mout, j, mo]
    wg28 = nc.dram_tensor("wg28", [128, 2, 2, 128], F8, kind="ExternalInput").ap()
    bu = nc.dram_tensor("bu", [128, D // 128], F32, kind="ExternalInput").ap()
    bgn = nc.dram_tensor("bgn", [128, D // 128], F32, kind="ExternalInput").ap()
    outT = nc.dram_tensor("outT", [D, P], F16, kind="ExternalOutput").ap()

    es = ExitStack()
    with tile.TileContext(nc) as tc:
        cpool = es.enter_context(tc.tile_pool(name="const", bufs=1))
        pool = es.enter_context(tc.tile_pool(name="sbuf", bufs=4))
        psum = es.enter_context(tc.tile_pool(name="psum", bufs=1, space="PSUM"))

        # constants load via the scalar engine's queue so the sync queue
        # can start streaming the first data tiles immediately
        wu_sb = []
        for k in range(2):
            t = cpool.tile([128, D], F16, tag=f"wu{k}")
            nc.scalar.dma_start(t[:], wu[128 * k : 128 * (k + 1), :])
            wu_sb.append(t)
        wgn_sb = []
        for k in range(4):
            t = cpool.tile([128, D], F16, tag=f"wgn{k}")
            nc.scalar.dma_start(t[:], wgn[128 * k : 128 * (k + 1), :])
            wgn_sb.append(t)
        wg28_sb = cpool.tile([128, 2, 2, 128], F8, tag="wg28")
        nc.scalar.dma_start(wg28_sb[:], wg28[:])
        bu_sb = cpool.tile([128, D // 128], F32)
        nc.scalar.dma_start(bu_sb[:], bu[:])
        bgn_sb = cpool.tile([128, D // 128], F32)
        nc.scalar.dma_start(bgn_sb[:], bgn[:])

        def supertile(c0, w):
            """Process cols [c0, c0+w); w multiple of 1024."""
            nh = w // 512
            CT = []
            for k in range(2):
                ct = pool.tile([128, w], F16, tag=f"ct{k}")
                nc.sync.dma_start(ct[:], ctxT[128 * k : 128 * (k + 1), c0 : c0 + w])
                CT.append(ct)
            PT = []
            for k in range(2):
                p = pool.tile([128, w], F16, tag=f"pt{k}")
                nc.sync.dma_start(p[:], prevT[128 * k : 128 * (k + 1), c0 : c0 + w])
                PT.append(p)
            UT = pool.tile([128, 2, w], FU, tag="ut")
            ZP = pool.tile([128, 2, w], F16, tag="zp")
            for h in range(nh):
                hs = slice(512 * h, 512 * (h + 1))
                ups = psum.tile([128, 2, 512], F32, tag=f"u{h % 2}")
                for m in range(2):
                    for k in range(2):
                        nc.tensor.matmul(
                            out=ups[:, m, :],
                            lhsT=wu_sb[k][:, 128 * m : 128 * (m + 1)],
                            rhs=CT[k][:, hs],
                            start=(k == 0),
                            stop=(k == 1),
                        )
                if zero_bias:
                    nc.scalar.activation(UT[:, :, hs], ups[:], AF.Tanh)
                else:
                    for m in range(2):
                        nc.scalar.activation(
                            UT[:, m, hs], ups[:, m, :], AF.Tanh,
                            bias=bu_sb[:, m : m + 1],
                        )
            for h in range(nh):
                hs = slice(512 * h, 512 * (h + 1))
                zps = psum.tile([128, 2, 512], F32, tag=f"z{h % 2}")
                for m in range(2):
                    for k in range(2):
                        nc.tensor.matmul(
                            out=zps[:, m, :],
                            lhsT=wgn_sb[k][:, 128 * m : 128 * (m + 1)],
                            rhs=PT[k][:, hs],
                            start=(k == 0),
                            stop=False,
                        )
                    if u_fp8:
                        nc.tensor.matmul(
                            out=zps[:, m, :],
                            lhsT=wg28_sb[:, m],
                            rhs=UT[:, :, hs],
                            start=False,
                            stop=True,
                            perf_mode=DR,
                        )
                    else:
                        for k in range(2):
                            nc.tensor.matmul(
                                out=zps[:, m, :],
                                lhsT=wgn_sb[2 + k][:, 128 * m : 128 * (m + 1)],
                                rhs=UT[:, k, hs],
                                start=False,
                                stop=(k == 1),
                            )
                if zero_bias:
                    nc.scalar.activation(ZP[:, :, hs], zps[:], AF.Sigmoid)
                else:
                    for m in range(2):
                        nc.scalar.activation(
                            ZP[:, m, hs], zps[:, m, :], AF.Sigmoid,
                            bias=bgn_sb[:, m : m + 1],
                        )
            for k in range(2):
                o = pool.tile([128, w], F16, tag=f"o{k}")
                nc.vector.tensor_sub(o[:], UT[:, k, :], PT[k][:])
                nc.vector.tensor_mul(o[:], o[:], ZP[:, k, :])
                nc.sync.dma_start(outT[128 * k : 128 * (k + 1), c0 : c0 + w], o[:])

        # narrow head and tapered tail keep the pipeline fill/drain short
        supertile(0, 512)
        supertile(512, 512)
        for s in range((P - 3 * 1024) // W):
            supertile(1024 + W * s, W)
        supertile(P - 2 * 1024, 1024)
        supertile(P - 1024, 512)
        supertile(P - 512, 512)
        es.close()
    nc.compile()
    _cache[key] = nc
    return nc


def _prep(prev, cfg, map_key, map_val, W_update, b_update, W_gate, b_gate):
    """Host-side shard prep: pad/split pairs, gather to dense fp16 arrays."""
    from concourse import mybir

    F8NP = mybir.dt.np(mybir.dt.float8e4)
    prev = np.ascontiguousarray(prev, dtype=np.float32)
    cfg = np.ascontiguousarray(cfg, dtype=np.float32)

    total = NCORES * P
    key_pad = np.zeros(total, np.int32)
    val_pad = np.zeros(total, np.int32)
    key_pad[:M] = map_key
    val_pad[:M] = map_val

    cfg16 = cfg.astype(np.float16)
    prev16 = prev.astype(np.float16)
    wu16 = np.ascontiguousarray(W_update.astype(np.float16))
    wgn = -np.asarray(W_gate, dtype=np.float32)
    wgn16 = np.ascontiguousarray(wgn.astype(np.float16))
    # DoubleRow fp8 u-side gate weights: [ki, mout, j, mo], feature = j*128 + ki
    wg28 = np.ascontiguousarray(
        wgn[256:].reshape(2, 128, 2, 128).transpose(1, 2, 0, 3)
    ).astype(F8NP)
    bu2 = np.ascontiguousarray(b_update.reshape(2, 128).T, dtype=np.float32)
    bgn2 = np.ascontiguousarray((-b_gate).reshape(2, 128).T, dtype=np.float32)
    zero_bias = not (np.any(b_update) or np.any(b_gate))

    in_maps = []
    for c in range(NCORES):
        ks = key_pad[c * P : (c + 1) * P]
        vs = val_pad[c * P : (c + 1) * P]
        in_maps.append(
            {
                "prevT": np.ascontiguousarray(prev16[ks].T),
                "ctxT": np.ascontiguousarray(cfg16[vs].T),
                "wu": wu16,
                "wgn": wgn16,
                "wg28": wg28,
                "bu": bu2,
                "bgn": bgn2,
            }
        )
    return in_maps, key_pad, zero_bias


def kernel(
    previous_ast_nodes_encodings,
    new_cfg_nodes_encodings,
    map_key_indices,
    map_val_indices,
    W_update,
    b_update,
    W_gate,
    b_gate,
):
    prev = np.asarray(previous_ast_nodes_encodings)
    in_maps, key_pad, zero_bias = _prep(
        prev,
        np.asarray(new_cfg_nodes_encodings),
        np.asarray(map_key_indices),
        np.asarray(map_val_indices),
        np.asarray(W_update),
        np.asarray(b_update),
        np.asarray(W_gate),
        np.asarray(b_gate),
    )
    nc = _build(zero_bias, U_FP8)

    from concourse import bass2jax

    def run_once():
        profile_dir = os.environ.get("KERNEL_PROFILE_DIR") or None
        if profile_dir is None:
            results = bass2jax.run_bass_via_pjrt(nc, in_maps, n_cores=NCORES)
        else:
            from trn_agent_boot.trn_boot import _ntff_profile_via_ctypes

            hook = _ntff_profile_via_ctypes("/opt/axon/libaxon_pjrt.so")
            os.makedirs(profile_dir, exist_ok=True)
            with hook(profile_dir, list(range(NCORES))):
                results = bass2jax.run_bass_via_pjrt(nc, in_maps, n_cores=NCORES)
        delta = np.empty((M, D), np.float32)
        for c in range(NCORES):
            lo, hi = c * P, min((c + 1) * P, M)
            if lo >= M:
                break
            oT = results[c]["outT"]
            delta[lo:hi] = oT[:, : hi - lo].T
        return delta

    # rare transient device glitches can produce non-finite outputs;
    # one clean retry recovers them
    delta = run_once()
    for _ in range(2):
        if np.isfinite(delta).all():
            break
        delta = run_once()

    out = prev.astype(np.float32, copy=True)
    keys = key_pad[:M]
    out[keys] = prev[keys] + delta
    return out


# revision 16
# speedup vs baseline: 1.0592x; 1.0592x over previous
"""Trainium2 Bass kernel for nn_CodeExpressionContextMixer.

Computes, for a mapping (key -> val) over AST/CFG node tables:
    u   = tanh(cfg[val] @ W_update + b_update)
    z   = sigmoid(prev[key] @ Wg1 + u @ Wg2 + b_gate)
    out = prev.at[key].set(z * prev[key] + (1 - z) * u)

Strategy (8 NeuronCores, SPMD, no collectives):
  * The index shuffle is host-side prep: pairs are split evenly across
    cores and the host gathers prev[key] / cfg[val] into dense fp16
    feature-major arrays per core.  The device kernel is pure dense
    streaming: no on-device gathers, no masks - the host scatters the
    (disjoint) target rows back and passes unmapped rows through
    exactly.
  * u matmuls and the prev side of the gate run fp16.  The u side of
    the K=512 gate matmul runs fp8e4m3 with DoubleRow (2 weights per
    PE cell, one pass instead of two): the tanh activation writes u
    as fp8 directly, so the cast is free.
  * Gate weights/bias are negated so ACT computes zp = 1-z =
    sigmoid(-arg); the device ships delta = zp*(u-p) in fp16 and the
    host adds the f32 prev rows back (better precision than routing
    prev through fp16, and one less DVE op per tile).
"""

import os
import numpy as np

R = 500000          # AST rows
CFGN = 100000       # CFG rows
D = 256             # feature dim
NCORES = 8
M = 400000          # mapped pairs
P = 50176           # padded pairs per core (49 * 1024)
W = 2048            # steady-state supertile width (cols)
U_FP8 = True        # u as fp8 (DoubleRow gate) vs all-fp16 fallback

_cache = {}


def _build(zero_bias, u_fp8):
    key = (zero_bias, u_fp8)
    if key in _cache:
        return _cache[key]
    from contextlib import ExitStack
    import concourse.bass as bass
    import concourse.tile as tile
    from concourse import bacc, mybir

    F32 = mybir.dt.float32
    F16 = mybir.dt.float16
    F8 = mybir.dt.float8e4
    FU = F8 if u_fp8 else F16
    AF = mybir.ActivationFunctionType
    DR = mybir.MatmulPerfMode.DoubleRow

    nc = bacc.Bacc("TRN2", target_bir_lowering=False, debug=False)

    prevT = nc.dram_tensor("prevT", [D, P], F16, kind="ExternalInput").ap()
    ctxT = nc.dram_tensor("ctxT", [D, P], F16, kind="ExternalInput").ap()
    wu = nc.dram_tensor("wu", [D, D], F16, kind="ExternalInput").ap()
    wgn = nc.dram_tensor("wgn", [2 * D, D], F16, kind="ExternalInput").ap()
    # fp8 u-side gate weights, DoubleRow layout [ki, mout, j, mo]
    wg28 = nc.dram_tensor("wg28", [128, 2, 2, 128], F8, kind="ExternalInput").ap()
    bu = nc.dram_tensor("bu", [128, D // 128], F32, kind="ExternalInput").ap()
    bgn = nc.dram_tensor("bgn", [128, D // 128], F32, kind="ExternalInput").ap()
    outT = nc.dram_tensor("outT", [D, P], F16, kind="ExternalOutput").ap()

    es = ExitStack()
    with tile.TileContext(nc) as tc:
        cpool = es.enter_context(tc.tile_pool(name="const", bufs=1))
        pool = es.enter_context(tc.tile_pool(name="sbuf", bufs=4))
        psum = es.enter_context(tc.tile_pool(name="psum", bufs=1, space="PSUM"))

        # constants load via the scalar engine's queue so the sync queue
        # can start streaming the first data tiles immediately
        wu_sb = []
        for k in range(2):
            t = cpool.tile([128, D], F16, tag=f"wu{k}")
            nc.scalar.dma_start(t[:], wu[128 * k : 128 * (k + 1), :])
            wu_sb.append(t)
        wgn_sb = []
        for k in range(4):
            t = cpool.tile([128, D], F16, tag=f"wgn{k}")
            nc.scalar.dma_start(t[:], wgn[128 * k : 128 * (k + 1), :])
            wgn_sb.append(t)
        wg28_sb = cpool.tile([128, 2, 2, 128], F8, tag="wg28")
        nc.scalar.dma_start(wg28_sb[:], wg28[:])
        bu_sb = cpool.tile([128, D // 128], F32)
        nc.scalar.dma_start(bu_sb[:], bu[:])
        bgn_sb = cpool.tile([128, D // 128], F32)
        nc.scalar.dma_start(bgn_sb[:], bgn[:])

        def supertile(c0, w):
            """Process cols [c0, c0+w); w multiple of 1024."""
            nh = w // 512
            CT = []
            for k in range(2):
                ct = pool.tile([128, w], F16, tag=f"ct{k}")
                nc.sync.dma_start(ct[:], ctxT[128 * k : 128 * (k + 1), c0 : c0 + w])
                CT.append(ct)
            PT = []
            for k in range(2):
                p = pool.tile([128, w], F16, tag=f"pt{k}")
                nc.sync.dma_start(p[:], prevT[128 * k : 128 * (k + 1), c0 : c0 + w])
                PT.append(p)
            UT = pool.tile([128, 2, w], FU, tag="ut")
            ZP = pool.tile([128, 2, w], F16, tag="zp")
            for h in range(nh):
                hs = slice(512 * h, 512 * (h + 1))
                ups = psum.tile([128, 2, 512], F32, tag=f"u{h % 2}")
                for m in range(2):
                    for k in range(2):
                        nc.tensor.matmul(
                            out=ups[:, m, :],
                            lhsT=wu_sb[k][:, 128 * m : 128 * (m + 1)],
                            rhs=CT[k][:, hs],
                            start=(k == 0),
                            stop=(k == 1),
                        )
                if zero_bias:
                    nc.scalar.activation(UT[:, :, hs], ups[:], AF.Tanh)
                else:
                    for m in range(2):
                        nc.scalar.activation(
                            UT[:, m, hs], ups[:, m, :], AF.Tanh,
                            bias=bu_sb[:, m : m + 1],
                        )
            for h in range(nh):
                hs = slice(512 * h, 512 * (h + 1))
                zps = psum.tile([128, 2, 512], F32, tag=f"z{h % 2}")
                for m in range(2):
                    for k in range(2):
                        nc.tensor.matmul(
                            out=zps[:, m, :],
                            lhsT=wgn_sb[k][:, 128 * m : 128 * (m + 1)],
                            rhs=PT[k][:, hs],
                            start=(k == 0),
                            stop=False,
                        )
                    if u_fp8:
                        nc.tensor.matmul(
                            out=zps[:, m, :],
                            lhsT=wg28_sb[:, m],
                            rhs=UT[:, :, hs],
                            start=False,
                            stop=True,
                            perf_mode=DR,
                        )
                    else:
                        for k in range(2):
                            nc.tensor.matmul(
                                out=zps[:, m, :],
                                lhsT=wgn_sb[2 + k][:, 128 * m : 128 * (m + 1)],
                                rhs=UT[:, k, hs],
                                start=False,
                                stop=(k == 1),
                            )
                if zero_bias:
                    nc.scalar.activation(ZP[:, :, hs], zps[:], AF.Sigmoid)
                else:
                    for m in range(2):
                        nc.scalar.activation(
                            ZP[:, m, hs], zps[:, m, :], AF.Sigmoid,
                            bias=bgn_sb[:, m : m + 1],
                        )
            for k in range(2):
                o = pool.tile([128, w], F16, tag=f"o{k}")
                nc.vector.tensor_sub(o[:], UT[:, k, :], PT[k][:])
                nc.vector.tensor_mul(o[:], o[:], ZP[:, k, :])
                nc.gpsimd.dma_start(outT[128 * k : 128 * (k + 1), c0 : c0 + w], o[:])

        # narrow head and tapered tail keep the pipeline fill/drain short
        supertile(0, 512)
        supertile(512, 512)
        for s in range((P - 3 * 1024) // W):
            supertile(1024 + W * s, W)
        supertile(P - 2 * 1024, 1024)
        supertile(P - 1024, 512)
        supertile(P - 512, 512)
        es.close()
    nc.compile()
    _cache[key] = nc
    return nc


def _prep(prev, cfg, map_key, map_val, W_update, b_update, W_gate, b_gate):
    """Host-side shard prep: pad/split pairs, gather to dense fp16 arrays."""
    from concourse import mybir

    F8NP = mybir.dt.np(mybir.dt.float8e4)
    prev = np.ascontiguousarray(prev, dtype=np.float32)
    cfg = np.ascontiguousarray(cfg, dtype=np.float32)

    total = NCORES * P
    key_pad = np.zeros(total, np.int32)
    val_pad = np.zeros(total, np.int32)
    key_pad[:M] = map_key
    val_pad[:M] = map_val

    cfg16 = cfg.astype(np.float16)
    prev16 = prev.astype(np.float16)
    wu16 = np.ascontiguousarray(W_update.astype(np.float16))
    wgn = -np.asarray(W_gate, dtype=np.float32)
    wgn16 = np.ascontiguousarray(wgn.astype(np.float16))
    # DoubleRow fp8 u-side gate weights: [ki, mout, j, mo], feature = j*128 + ki
    wg28 = np.ascontiguousarray(
        wgn[256:].reshape(2, 128, 2, 128).transpose(1, 2, 0, 3)
    ).astype(F8NP)
    bu2 = np.ascontiguousarray(b_update.reshape(2, 128).T, dtype=np.float32)
    bgn2 = np.ascontiguousarray((-b_gate).reshape(2, 128).T, dtype=np.float32)
    zero_bias = not (np.any(b_update) or np.any(b_gate))

    in_maps = []
    for c in range(NCORES):
        ks = key_pad[c * P : (c + 1) * P]
        vs = val_pad[c * P : (c + 1) * P]
        in_maps.append(
            {
                "prevT": np.ascontiguousarray(prev16[ks].T),
                "ctxT": np.ascontiguousarray(cfg16[vs].T),
                "wu": wu16,
                "wgn": wgn16,
                "wg28": wg28,
                "bu": bu2,
                "bgn": bgn2,
            }
        )
    return in_maps, key_pad, zero_bias


def kernel(
    previous_ast_nodes_encodings,
    new_cfg_nodes_encodings,
    map_key_indices,
    map_val_indices,
    W_update,
    b_update,
    W_gate,
    b_gate,
):
    prev = np.asarray(previous_ast_nodes_encodings)
    in_maps, key_pad, zero_bias = _prep(
        prev,
        np.asarray(new_cfg_nodes_encodings),
        np.asarray(map_key_indices),
        np.asarray(map_val_indices),
        np.asarray(W_update),
        np.asarray(b_update),
        np.asarray(W_gate),
        np.asarray(b_gate),
    )
    nc = _build(zero_bias, U_FP8)

    from concourse import bass2jax

    def run_once():
        profile_dir = os.environ.get("KERNEL_PROFILE_DIR") or None
        if profile_dir is None:
            results = bass2jax.run_bass_via_pjrt(nc, in_maps, n_cores=NCORES)
        else:
            from trn_agent_boot.trn_boot import _ntff_profile_via_ctypes

            hook = _ntff_profile_via_ctypes("/opt/axon/libaxon_pjrt.so")
            os.makedirs(profile_dir, exist_ok=True)
            with hook(profile_dir, list(range(NCORES))):
                results = bass2jax.run_bass_via_pjrt(nc, in_maps, n_cores=NCORES)
        delta = np.empty((M, D), np.float32)
        for c in range(NCORES):
            lo, hi = c * P, min((c + 1) * P, M)
            if lo >= M:
                break
            oT = results[c]["outT"]
            delta[lo:hi] = oT[:, : hi - lo].T
        return delta

    # rare transient device glitches can produce non-finite outputs;
    # one clean retry recovers them
    delta = run_once()
    for _ in range(2):
        if np.isfinite(delta).all():
            break
        delta = run_once()

    out = prev.astype(np.float32, copy=True)
    keys = key_pad[:M]
    out[keys] = prev[keys] + delta
    return out


# revision 19
# speedup vs baseline: 1.1227x; 1.0599x over previous
"""Trainium2 Bass kernel for nn_CodeExpressionContextMixer.

Computes, for a mapping (key -> val) over AST/CFG node tables:
    u   = tanh(cfg[val] @ W_update + b_update)
    z   = sigmoid(prev[key] @ Wg1 + u @ Wg2 + b_gate)
    out = prev.at[key].set(z * prev[key] + (1 - z) * u)

Strategy (8 NeuronCores, SPMD, no collectives):
  * The index shuffle is host-side prep: pairs are split evenly across
    cores and the host gathers prev[key] / cfg[val] into dense fp16
    feature-major arrays per core.  The device kernel is pure dense
    streaming: no on-device gathers, no masks - the host scatters the
    (disjoint) target rows back and passes unmapped rows through
    exactly.
  * u matmuls and the prev side of the gate run fp16.  The u side of
    the K=512 gate matmul runs fp8e4m3 with DoubleRow (2 weights per
    PE cell, one pass instead of two): the tanh activation writes u
    as fp8 directly, so the cast is free.
  * Gate weights/bias are negated so ACT computes zp = 1-z =
    sigmoid(-arg); the device ships delta = zp*(u-p) in fp16 and the
    host adds the f32 prev rows back (better precision than routing
    prev through fp16, and one less DVE op per tile).
"""

import os
import numpy as np

R = 500000          # AST rows
CFGN = 100000       # CFG rows
D = 256             # feature dim
NCORES = 8
M = 400000          # mapped pairs
P = 50176           # padded pairs per core (49 * 1024)
W = 2048            # steady-state supertile width (cols)
U_FP8 = True        # u as fp8 (DoubleRow gate) vs all-fp16 fallback

_cache = {}


def _build(zero_bias, u_fp8):
    key = (zero_bias, u_fp8)
    if key in _cache:
        return _cache[key]
    from contextlib import ExitStack
    import concourse.bass as bass
    import concourse.tile as tile
    from concourse import bacc, mybir

    F32 = mybir.dt.float32
    F16 = mybir.dt.float16
    F8 = mybir.dt.float8e4
    FU = F8 if u_fp8 else F16
    AF = mybir.ActivationFunctionType
    DR = mybir.MatmulPerfMode.DoubleRow

    nc = bacc.Bacc("TRN2", target_bir_lowering=False, debug=False)

    prevT = nc.dram_tensor("prevT", [D, P], F16, kind="ExternalInput").ap()
    ctxT = nc.dram_tensor("ctxT", [D, P], F16, kind="ExternalInput").ap()
    wu = nc.dram_tensor("wu", [D, D], F16, kind="ExternalInput").ap()
    wgn = nc.dram_tensor("wgn", [2 * D, D], F16, kind="ExternalInput").ap()
    # fp8 u-side gate weights, DoubleRow layout [ki, mout, j, mo]
    wg28 = nc.dram_tensor("wg28", [128, 2, 2, 128], F8, kind="ExternalInput").ap()
    bu = nc.dram_tensor("bu", [128, D // 128], F32, kind="ExternalInput").ap()
    bgn = nc.dram_tensor("bgn", [128, D // 128], F32, kind="ExternalInput").ap()
    outT = nc.dram_tensor("outT", [D, P], F16, kind="ExternalOutput").ap()

    es = ExitStack()
    with tile.TileContext(nc) as tc:
        cpool = es.enter_context(tc.tile_pool(name="const", bufs=1))
        pool = es.enter_context(tc.tile_pool(name="sbuf", bufs=4))
        psum = es.enter_context(tc.tile_pool(name="psum", bufs=1, space="PSUM"))

        # constants load via the scalar engine's queue so the sync queue
        # can start streaming the first data tiles immediately
        wu_sb = []
        for k in range(2):
            t = cpool.tile([128, D], F16, tag=f"wu{k}")
            nc.scalar.dma_start(t[:], wu[128 * k : 128 * (k + 1), :])
            wu_sb.append(t)
        wgn_sb = []
        for k in range(4):
            t = cpool.tile([128, D], F16, tag=f"wgn{k}")
            nc.scalar.dma_start(t[:], wgn[128 * k : 128 * (k + 1), :])
            wgn_sb.append(t)
        wg28_sb = cpool.tile([128, 2, 2, 128], F8, tag="wg28")
        nc.scalar.dma_start(wg28_sb[:], wg28[:])
        bu_sb = cpool.tile([128, D // 128], F32)
        nc.scalar.dma_start(bu_sb[:], bu[:])
        bgn_sb = cpool.tile([128, D // 128], F32)
        nc.scalar.dma_start(bgn_sb[:], bgn[:])

        def supertile(c0, w, store_engine=None):
            """Process cols [c0, c0+w); w multiple of 512."""
            store_engine = store_engine or nc.gpsimd
            nh = w // 512
            CT = []
            for k in range(2):
                ct = pool.tile([128, w], F16, tag=f"ct{k}")
                nc.sync.dma_start(ct[:], ctxT[128 * k : 128 * (k + 1), c0 : c0 + w])
                CT.append(ct)
            PT = []
            for k in range(2):
                p = pool.tile([128, w], F16, tag=f"pt{k}")
                nc.sync.dma_start(p[:], prevT[128 * k : 128 * (k + 1), c0 : c0 + w])
                PT.append(p)
            UT = pool.tile([128, 2, w], FU, tag="ut")
            ZP = pool.tile([128, 2, w], F16, tag="zp")
            for h in range(nh):
                hs = slice(512 * h, 512 * (h + 1))
                ups = psum.tile([128, 2, 512], F32, tag=f"u{h % 2}")
                for m in range(2):
                    for k in range(2):
                        nc.tensor.matmul(
                            out=ups[:, m, :],
                            lhsT=wu_sb[k][:, 128 * m : 128 * (m + 1)],
                            rhs=CT[k][:, hs],
                            start=(k == 0),
                            stop=(k == 1),
                        )
                if zero_bias:
                    nc.scalar.activation(UT[:, :, hs], ups[:], AF.Tanh)
                else:
                    for m in range(2):
                        nc.scalar.activation(
                            UT[:, m, hs], ups[:, m, :], AF.Tanh,
                            bias=bu_sb[:, m : m + 1],
                        )
            for h in range(nh):
                hs = slice(512 * h, 512 * (h + 1))
                zps = psum.tile([128, 2, 512], F32, tag=f"z{h % 2}")
                for m in range(2):
                    for k in range(2):
                        nc.tensor.matmul(
                            out=zps[:, m, :],
                            lhsT=wgn_sb[k][:, 128 * m : 128 * (m + 1)],
                            rhs=PT[k][:, hs],
                            start=(k == 0),
                            stop=False,
                        )
                    if u_fp8:
                        nc.tensor.matmul(
                            out=zps[:, m, :],
                            lhsT=wg28_sb[:, m],
                            rhs=UT[:, :, hs],
                            start=False,
                            stop=True,
                            perf_mode=DR,
                        )
                    else:
                        for k in range(2):
                            nc.tensor.matmul(
                                out=zps[:, m, :],
                                lhsT=wgn_sb[2 + k][:, 128 * m : 128 * (m + 1)],
                                rhs=UT[:, k, hs],
                                start=False,
                                stop=(k == 1),
                            )
                if zero_bias:
                    nc.scalar.activation(ZP[:, :, hs], zps[:], AF.Sigmoid)
                else:
                    for m in range(2):
                        nc.scalar.activation(
                            ZP[:, m, hs], zps[:, m, :], AF.Sigmoid,
                            bias=bgn_sb[:, m : m + 1],
                        )
            for k in range(2):
                o = pool.tile([128, w], F16, tag=f"o{k}")
                nc.vector.tensor_sub(o[:], UT[:, k, :], PT[k][:])
                nc.vector.tensor_mul(o[:], o[:], ZP[:, k, :])
                store_engine.dma_start(outT[128 * k : 128 * (k + 1), c0 : c0 + w], o[:])

        # narrow head and tapered tail keep the pipeline fill/drain short
        supertile(0, 512)
        supertile(512, 512)
        for s in range((P - 3 * 1024) // W):
            supertile(1024 + W * s, W)
        supertile(P - 2 * 1024, 1024)
        supertile(P - 1024, 512, store_engine=nc.scalar)
        supertile(P - 512, 512, store_engine=nc.scalar)
        es.close()
    nc.compile()
    _cache[key] = nc
    return nc


def _prep(prev, cfg, map_key, map_val, W_update, b_update, W_gate, b_gate):
    """Host-side shard prep: pad/split pairs, gather to dense fp16 arrays."""
    from concourse import mybir

    F8NP = mybir.dt.np(mybir.dt.float8e4)
    prev = np.ascontiguousarray(prev, dtype=np.float32)
    cfg = np.ascontiguousarray(cfg, dtype=np.float32)

    total = NCORES * P
    key_pad = np.zeros(total, np.int32)
    val_pad = np.zeros(total, np.int32)
    key_pad[:M] = map_key
    val_pad[:M] = map_val

    cfg16 = cfg.astype(np.float16)
    prev16 = prev.astype(np.float16)
    wu16 = np.ascontiguousarray(W_update.astype(np.float16))
    wgn = -np.asarray(W_gate, dtype=np.float32)
    wgn16 = np.ascontiguousarray(wgn.astype(np.float16))
    # DoubleRow fp8 u-side gate weights: [ki, mout, j, mo], feature = j*128 + ki
    wg28 = np.ascontiguousarray(
        wgn[256:].reshape(2, 128, 2, 128).transpose(1, 2, 0, 3)
    ).astype(F8NP)
    bu2 = np.ascontiguousarray(b_update.reshape(2, 128).T, dtype=np.float32)
    bgn2 = np.ascontiguousarray((-b_gate).reshape(2, 128).T, dtype=np.float32)
    zero_bias = not (np.any(b_update) or np.any(b_gate))

    in_maps = []
    for c in range(NCORES):
        ks = key_pad[c * P : (c + 1) * P]
        vs = val_pad[c * P : (c + 1) * P]
        in_maps.append(
            {
                "prevT": np.ascontiguousarray(prev16[ks].T),
                "ctxT": np.ascontiguousarray(cfg16[vs].T),
                "wu": wu16,
                "wgn": wgn16,
                "wg28": wg28,
                "bu": bu2,
                "bgn": bgn2,
            }
        )
    return in_maps, key_pad, zero_bias


def kernel(
    previous_ast_nodes_encodings,
    new_cfg_nodes_encodings,
    map_key_indices,
    map_val_indices,
    W_update,
    b_update,
    W_gate,
    b_gate,
):
    prev = np.asarray(previous_ast_nodes_encodings)
    in_maps, key_pad, zero_bias = _prep(
        prev,
        np.asarray(new_cfg_nodes_encodings),
        np.asarray(map_key_indices),
        np.asarray(map_val_indices),
        np.asarray(W_update),
        np.asarray(b_update),
        np.asarray(W_gate),
        np.asarray(b_gate),
    )
    nc = _build(zero_bias, U_FP8)

    from concourse import bass2jax

    def run_once():
        profile_dir = os.environ.get("KERNEL_PROFILE_DIR") or None
        if profile_dir is None:
            results = bass2jax.run_bass_via_pjrt(nc, in_maps, n_cores=NCORES)
        else:
            from trn_agent_boot.trn_boot import _ntff_profile_via_ctypes

            hook = _ntff_profile_via_ctypes("/opt/axon/libaxon_pjrt.so")
            os.makedirs(profile_dir, exist_ok=True)
            with hook(profile_dir, list(range(NCORES))):
                results = bass2jax.run_bass_via_pjrt(nc, in_maps, n_cores=NCORES)
        delta = np.empty((M, D), np.float32)
        for c in range(NCORES):
            lo, hi = c * P, min((c + 1) * P, M)
            if lo >= M:
                break
            oT = results[c]["outT"]
            delta[lo:hi] = oT[:, : hi - lo].T
        return delta

    # rare transient device glitches can produce non-finite outputs;
    # one clean retry recovers them
    delta = run_once()
    for _ in range(2):
        if np.isfinite(delta).all():
            break
        delta = run_once()

    out = prev.astype(np.float32, copy=True)
    keys = key_pad[:M]
    out[keys] = prev[keys] + delta
    return out
